# revision 2
# baseline (speedup 1.0000x reference)
"""Causal attention (QKV proj + softmax + PV + ReLU) on 8 trn2 NeuronCores.

Sharding: data-parallel over batch B=32 -> 4 batches per core; projection
weights replicated. Inside each core, per batch:
  X^T via DMA-transpose (bf16)
  Q^T,K^T = W^T.T @ X^T   (bias folded into PSUM->SBUF drain)
  V = X^T.T @ Wv^T        (bias via DVE add during drain)
  S^T[j,i] tiles = K^T.T @ Q^T, causal-sparse (upper j>i tiles skipped)
  P^T = exp(scale*S^T + padmask_bias_j)  (ACT, fused), diag tile masked by mul
  O' = P^T.T @ V and rowsum = P^T.T @ ones ride the same stationary operand
  out = Relu(O' * (1/rowsum))  (one ACT op), DMA out.
"""

import os
from contextlib import ExitStack

import numpy as np
import ml_dtypes

import concourse.bass as bass
import concourse.tile as tile
from concourse import bacc, mybir
from concourse import bass_utils

F32 = mybir.dt.float32
BF16 = mybir.dt.bfloat16
AF = mybir.ActivationFunctionType

N_CORES = 8
B = 32
L = 1024
C = 1024  # d_model
D = 512
P = 128
NB = B // N_CORES  # batches per core
CT = C // P  # 8 contraction tiles
DT = D // P  # 4 d tiles
LT = L // P  # 8 l/j/i tiles
SCALE = float(D) ** -0.5
NEG = -30000.0


def build_program(nb: int = NB):
    """Build the per-core Bass program for nb batches."""
    nc = bacc.Bacc("TRN2", target_bir_lowering=False, debug=False,
                   num_devices=N_CORES)

    xb = nc.dram_tensor("xb", [nb, L, C], BF16, kind="ExternalInput").ap()
    wqT = nc.dram_tensor("wqT", [C, D], BF16, kind="ExternalInput").ap()
    wkT = nc.dram_tensor("wkT", [C, D], BF16, kind="ExternalInput").ap()
    wvT = nc.dram_tensor("wvT", [C, D], BF16, kind="ExternalInput").ap()
    bq2 = nc.dram_tensor("bq2", [P, DT], F32, kind="ExternalInput").ap()
    bk2 = nc.dram_tensor("bk2", [P, DT], F32, kind="ExternalInput").ap()
    bvb = nc.dram_tensor("bvb", [P, D], F32, kind="ExternalInput").ap()
    pmt = nc.dram_tensor("pmt", [nb, P, LT], F32, kind="ExternalInput").ap()
    tri = nc.dram_tensor("tri", [P, P], BF16, kind="ExternalInput").ap()
    out = nc.dram_tensor("out", [nb, L, D], F32, kind="ExternalOutput").ap()

    with tile.TileContext(nc) as tc, ExitStack() as ctx:
        const = ctx.enter_context(tc.tile_pool(name="const", bufs=1))
        xt_pool = ctx.enter_context(tc.tile_pool(name="xt", bufs=2))
        qk_pool = ctx.enter_context(tc.tile_pool(name="qk", bufs=2))
        v_pool = ctx.enter_context(tc.tile_pool(name="v", bufs=2))
        pt_pool = ctx.enter_context(tc.tile_pool(name="pt", bufs=2))
        o_pool = ctx.enter_context(tc.tile_pool(name="o", bufs=3))
        sm_pool = ctx.enter_context(tc.tile_pool(name="sm", bufs=4))
        pm_pool = ctx.enter_context(tc.tile_pool(name="pm", bufs=2))
        proj_ps = ctx.enter_context(tc.tile_pool(name="pps", bufs=2, space="PSUM"))
        s_ps = ctx.enter_context(tc.tile_pool(name="sps", bufs=2, space="PSUM"))
        o_ps = ctx.enter_context(tc.tile_pool(name="ops", bufs=2, space="PSUM"))
        r_ps = ctx.enter_context(tc.tile_pool(name="rps", bufs=2, space="PSUM"))

        # --- constants, loaded once ---
        wq_sb = const.tile([P, CT, D], BF16)
        wk_sb = const.tile([P, CT, D], BF16)
        wv_sb = const.tile([P, CT, D], BF16)
        for w_sb, w_ap in ((wq_sb, wqT), (wk_sb, wkT), (wv_sb, wvT)):
            nc.sync.dma_start(w_sb[:], w_ap.rearrange("(t p) d -> p t d", p=P))
        bq_sb = const.tile([P, DT], F32)
        nc.sync.dma_start(bq_sb[:], bq2[:])
        bk_sb = const.tile([P, DT], F32)
        nc.sync.dma_start(bk_sb[:], bk2[:])
        bv_sb = const.tile([P, D], F32)
        nc.sync.dma_start(bv_sb[:], bvb[:])
        tri_sb = const.tile([P, P], BF16)
        nc.sync.dma_start(tri_sb[:], tri[:])
        ones_sb = const.tile([P, 1], BF16)
        nc.vector.memset(ones_sb[:], 1.0)

        for b in range(nb):
            # --- X^T: DMA-transpose x[b] into 8 [128c, 1024l] bf16 tiles ---
            xt = []
            for ct in range(CT):
                t = xt_pool.tile([P, L], BF16, tag=f"xt{ct}", name=f"xt{ct}_{b}")
                nc.sync.dma_start(t[:], xb[b, :, ct * P:(ct + 1) * P],
                                  transpose=True)
                xt.append(t)
            pm_sb = pm_pool.tile([P, LT], F32, name=f"pm_{b}")
            nc.sync.dma_start(pm_sb[:], pmt[b])

            # --- Q^T, K^T: [128d, 1024l] tiles; bias added at drain ---
            qt, kt = [], []
            for name, w_sb, b_sb, dst in (("q", wq_sb, bq_sb, qt),
                                          ("k", wk_sb, bk_sb, kt)):
                for dt in range(DT):
                    t = qk_pool.tile([P, L], BF16, tag=f"{name}t{dt}",
                                     name=f"{name}t{dt}_{b}")
                    dst.append(t)
                    for lc in range(L // 512):
                        ps = proj_ps.tile([P, 512], F32, tag="pp",
                                          name=f"{name}ps{dt}_{lc}_{b}")
                        for ct in range(CT):
                            nc.tensor.matmul(
                                ps[:],
                                w_sb[:, ct, dt * P:(dt + 1) * P],
                                xt[ct][:, lc * 512:(lc + 1) * 512],
                                start=(ct == 0), stop=(ct == CT - 1))
                        nc.vector.tensor_scalar_add(
                            t[:, lc * 512:(lc + 1) * 512], ps[:],
                            b_sb[:, dt:dt + 1])

            # --- V: [128l, 512d] tiles; bias via DVE add at drain ---
            v = []
            for lt in range(LT):
                t = v_pool.tile([P, D], BF16, tag=f"v{lt}", name=f"v{lt}_{b}")
                v.append(t)
                ps = proj_ps.tile([P, D], F32, tag="pp", name=f"vps{lt}_{b}")
                for ct in range(CT):
                    nc.tensor.matmul(ps[:], xt[ct][:, lt * P:(lt + 1) * P],
                                     wv_sb[:, ct, :],
                                     start=(ct == 0), stop=(ct == CT - 1))
                nc.vector.tensor_add(t[:], ps[:], bv_sb[:])

            # --- S^T tiles + exp -> P^T (causal: only i >= j0 computed) ---
            pt = []
            for jb in range(LT):
                j0 = jb * P
                t = pt_pool.tile([P, L], BF16, tag=f"pt{jb}", name=f"pt{jb}_{b}")
                pt.append(t)
                i0 = j0
                while i0 < L:
                    n = min((i0 // 512 + 1) * 512, L) - i0
                    ps = s_ps.tile([P, n], F32, tag="sp", name=f"sps{jb}_{i0}_{b}")
                    for dt in range(DT):
                        nc.tensor.matmul(ps[:], kt[dt][:, j0:j0 + P],
                                         qt[dt][:, i0:i0 + n],
                                         start=(dt == 0), stop=(dt == DT - 1))
                    nc.scalar.activation(t[:, i0:i0 + n], ps[:], AF.Exp,
                                         bias=pm_sb[:, jb:jb + 1], scale=SCALE)
                    i0 += n
                # mask the diagonal tile: keep j<=i (upper-right triangle)
                nc.vector.tensor_mul(t[:, j0:j0 + P], t[:, j0:j0 + P], tri_sb[:])

            # --- O' = P^T.T @ V, rowsum = P^T.T @ ones; normalize+relu ---
            for ib in range(LT):
                i0 = ib * P
                ops = o_ps.tile([P, D], F32, tag="op", name=f"ops{ib}_{b}")
                rps = r_ps.tile([P, 1], F32, tag="rp", name=f"rps{ib}_{b}")
                for jb in range(ib + 1):
                    pT = pt[jb][:, i0:i0 + P]
                    nc.tensor.matmul(ops[:], pT, v[jb][:],
                                     start=(jb == 0), stop=(jb == ib))
                    nc.tensor.matmul(rps[:], pT, ones_sb[:],
                                     start=(jb == 0), stop=(jb == ib))
                rec = sm_pool.tile([P, 1], F32, tag="rec", name=f"rec{ib}_{b}")
                nc.vector.reciprocal(rec[:], rps[:])
                o_sb = o_pool.tile([P, D], F32, tag="ot", name=f"o{ib}_{b}")
                nc.scalar.activation(o_sb[:], ops[:], AF.Relu, scale=rec[:])
                nc.sync.dma_start(out[b, i0:i0 + P, :], o_sb[:])

    nc.compile()
    return nc


def _prep_host(x, Wq, bq, Wk, bk, Wv, bv, mask):
    bf = ml_dtypes.bfloat16
    xb16 = np.ascontiguousarray(x).astype(bf)  # [B, L, C]
    wqT = np.ascontiguousarray(Wq.T).astype(bf)  # [C, D]
    wkT = np.ascontiguousarray(Wk.T).astype(bf)
    wvT = np.ascontiguousarray(Wv.T).astype(bf)
    bq2 = np.ascontiguousarray(
        bq.astype(np.float32).reshape(DT, P).T)  # [P, DT]
    bk2 = np.ascontiguousarray(bk.astype(np.float32).reshape(DT, P).T)
    bvb = np.ascontiguousarray(
        np.broadcast_to(bv.astype(np.float32), (P, D)))  # [P, D]
    pm = np.where(mask[:, 0, :] != 0, 0.0, NEG).astype(np.float32)  # [B, L]
    pmt = np.ascontiguousarray(
        pm.reshape(B, LT, P).transpose(0, 2, 1))  # [B, P, LT]
    tri = (np.arange(P)[:, None] <= np.arange(P)[None, :]).astype(bf)
    return xb16, wqT, wkT, wvT, bq2, bk2, bvb, pmt, tri


_NC_CACHE = {}


def kernel(x, Wq, bq, Wk, bk, Wv, bv, mask):
    x = np.asarray(x)
    Wq, bq = np.asarray(Wq), np.asarray(bq)
    Wk, bk = np.asarray(Wk), np.asarray(bk)
    Wv, bv = np.asarray(Wv), np.asarray(bv)
    mask = np.asarray(mask)

    xb16, wqT, wkT, wvT, bq2, bk2, bvb, pmt, tri = _prep_host(
        x, Wq, bq, Wk, bk, Wv, bv, mask)

    if "nc" not in _NC_CACHE:
        _NC_CACHE["nc"] = build_program(NB)
    nc = _NC_CACHE["nc"]

    in_maps = []
    for c in range(N_CORES):
        s = slice(c * NB, (c + 1) * NB)
        in_maps.append({
            "xb": np.ascontiguousarray(xb16[s]),
            "wqT": wqT, "wkT": wkT, "wvT": wvT,
            "bq2": bq2, "bk2": bk2, "bvb": bvb,
            "pmt": np.ascontiguousarray(pmt[s]),
            "tri": tri,
        })

    res = bass_utils.run_bass_kernel_spmd(
        nc, in_maps, core_ids=list(range(N_CORES)),
        trace=bool(int(os.environ.get("KERNEL_TRACE", "0"))),
    )
    if os.environ.get("KERNEL_RESULT_HOOK"):
        _NC_CACHE["last_result"] = res

    return np.concatenate([res.results[c]["out"] for c in range(N_CORES)],
                          axis=0)


# revision 7
# speedup vs baseline: 1.0760x; 1.0760x over previous
"""Causal attention (QKV proj + softmax + PV + ReLU) on 8 trn2 NeuronCores.

Sharding: data-parallel over batch B=32 -> 4 batches per core; projection
weights replicated. Inside each core, per batch:
  X^T via DMA-transpose (bf16)
  Q^T,K^T = W^T.T @ X^T   (bias folded into PSUM->SBUF drain)
  V = X^T.T @ Wv^T        (bias via DVE add during drain)
  S^T[j,i] tiles = K^T.T @ Q^T, causal-sparse (upper j>i tiles skipped)
  P^T = exp(scale*S^T + padmask_bias_j)  (ACT, fused), diag tile masked by mul
  O' = P^T.T @ V and rowsum = P^T.T @ ones ride the same stationary operand
  out = Relu(O' * (1/rowsum))  (one ACT op), DMA out.
"""

import os
from contextlib import ExitStack

import numpy as np
import ml_dtypes

import concourse.bass as bass
import concourse.tile as tile
from concourse import bacc, mybir
from concourse import bass_utils

F32 = mybir.dt.float32
BF16 = mybir.dt.bfloat16
AF = mybir.ActivationFunctionType

N_CORES = 8
B = 32
L = 1024
C = 1024  # d_model
D = 512
P = 128
NB = B // N_CORES  # batches per core
CT = C // P  # 8 contraction tiles
DT = D // P  # 4 d tiles
LT = L // P  # 8 l/j/i tiles
SCALE = float(D) ** -0.5
NEG = -30000.0


def build_program(nb: int = NB):
    """Build the per-core Bass program for nb batches."""
    nc = bacc.Bacc("TRN2", target_bir_lowering=False, debug=False,
                   num_devices=N_CORES)

    xtb = nc.dram_tensor("xtb", [nb, C, L], BF16, kind="ExternalInput").ap()
    wqT = nc.dram_tensor("wqT", [C, D], BF16, kind="ExternalInput").ap()
    wkT = nc.dram_tensor("wkT", [C, D], BF16, kind="ExternalInput").ap()
    wvT = nc.dram_tensor("wvT", [C, D], BF16, kind="ExternalInput").ap()
    bq2 = nc.dram_tensor("bq2", [P, DT], F32, kind="ExternalInput").ap()
    bk2 = nc.dram_tensor("bk2", [P, DT], F32, kind="ExternalInput").ap()
    bvb = nc.dram_tensor("bvb", [P, D], F32, kind="ExternalInput").ap()
    pmt = nc.dram_tensor("pmt", [nb, P, LT], F32, kind="ExternalInput").ap()
    tri = nc.dram_tensor("tri", [P, P], BF16, kind="ExternalInput").ap()
    out = nc.dram_tensor("out", [nb, L, D], F32, kind="ExternalOutput").ap()

    with tile.TileContext(nc) as tc, ExitStack() as ctx:
        const = ctx.enter_context(tc.tile_pool(name="const", bufs=1))
        xt_pool = ctx.enter_context(tc.tile_pool(name="xt", bufs=2))
        qk_pool = ctx.enter_context(tc.tile_pool(name="qk", bufs=2))
        v_pool = ctx.enter_context(tc.tile_pool(name="v", bufs=2))
        pt_pool = ctx.enter_context(tc.tile_pool(name="pt", bufs=2))
        o_pool = ctx.enter_context(tc.tile_pool(name="o", bufs=3))
        sm_pool = ctx.enter_context(tc.tile_pool(name="sm", bufs=4))
        pm_pool = ctx.enter_context(tc.tile_pool(name="pm", bufs=2))
        proj_ps = ctx.enter_context(tc.tile_pool(name="pps", bufs=2, space="PSUM"))
        s_ps = ctx.enter_context(tc.tile_pool(name="sps", bufs=2, space="PSUM"))
        o_ps = ctx.enter_context(tc.tile_pool(name="ops", bufs=2, space="PSUM"))
        r_ps = ctx.enter_context(tc.tile_pool(name="rps", bufs=2, space="PSUM"))

        # --- constants, loaded once; weights on the scalar HWDGE queue so
        # the sync queue is dedicated to x prefetch ---
        wq_sb = const.tile([P, CT, D], BF16)
        nc.sync.dma_start(wq_sb[:], wqT.rearrange("(t p) d -> p t d", p=P))
        wk_sb = const.tile([P, CT, D], BF16)
        nc.scalar.dma_start(wk_sb[:], wkT.rearrange("(t p) d -> p t d", p=P))
        wv_sb = const.tile([P, CT, D], BF16)
        nc.scalar.dma_start(wv_sb[:], wvT.rearrange("(t p) d -> p t d", p=P))
        bq_sb = const.tile([P, DT], F32)
        nc.scalar.dma_start(bq_sb[:], bq2[:])
        bk_sb = const.tile([P, DT], F32)
        nc.scalar.dma_start(bk_sb[:], bk2[:])
        bv_sb = const.tile([P, D], F32)
        nc.scalar.dma_start(bv_sb[:], bvb[:])
        tri_sb = const.tile([P, P], BF16)
        nc.scalar.dma_start(tri_sb[:], tri[:])
        ones_sb = const.tile([P, 1], BF16)
        nc.vector.memset(ones_sb[:], 1.0)

        for b in range(nb):
            # --- X^T tiles [128c, 1024l], pretransposed on host ---
            xt = []
            for ct in range(CT):
                t = xt_pool.tile([P, L], BF16, tag=f"xt{ct}", name=f"xt{ct}_{b}")
                nc.sync.dma_start(t[:], xtb[b, ct * P:(ct + 1) * P, :])
                xt.append(t)
            pm_sb = pm_pool.tile([P, LT], F32, name=f"pm_{b}")
            nc.sync.dma_start(pm_sb[:], pmt[b])

            # --- Q^T, K^T: [128d, 1024l] tiles; bias added at drain ---
            qt, kt = [], []
            for name, w_sb, b_sb, dst in (("q", wq_sb, bq_sb, qt),
                                          ("k", wk_sb, bk_sb, kt)):
                for dt in range(DT):
                    t = qk_pool.tile([P, L], BF16, tag=f"{name}t{dt}",
                                     name=f"{name}t{dt}_{b}")
                    dst.append(t)
                    for lc in range(L // 512):
                        ps = proj_ps.tile([P, 512], F32, tag="pp",
                                          name=f"{name}ps{dt}_{lc}_{b}")
                        for ct in range(CT):
                            nc.tensor.matmul(
                                ps[:],
                                w_sb[:, ct, dt * P:(dt + 1) * P],
                                xt[ct][:, lc * 512:(lc + 1) * 512],
                                start=(ct == 0), stop=(ct == CT - 1))
                        nc.vector.tensor_scalar_add(
                            t[:, lc * 512:(lc + 1) * 512], ps[:],
                            b_sb[:, dt:dt + 1])

            # --- V: [128l, 512d] tiles; bias via DVE add at drain ---
            v = []
            for lt in range(LT):
                t = v_pool.tile([P, D], BF16, tag=f"v{lt}", name=f"v{lt}_{b}")
                v.append(t)
                ps = proj_ps.tile([P, D], F32, tag="pp", name=f"vps{lt}_{b}")
                for ct in range(CT):
                    nc.tensor.matmul(ps[:], xt[ct][:, lt * P:(lt + 1) * P],
                                     wv_sb[:, ct, :],
                                     start=(ct == 0), stop=(ct == CT - 1))
                nc.vector.tensor_add(t[:], ps[:], bv_sb[:])

            # --- S^T tiles + exp -> P^T (causal: only i >= j0 computed) ---
            pt = []
            for jb in range(LT):
                j0 = jb * P
                t = pt_pool.tile([P, L], BF16, tag=f"pt{jb}", name=f"pt{jb}_{b}")
                pt.append(t)
                i0 = j0
                while i0 < L:
                    n = min((i0 // 512 + 1) * 512, L) - i0
                    ps = s_ps.tile([P, n], F32, tag="sp", name=f"sps{jb}_{i0}_{b}")
                    for dt in range(DT):
                        nc.tensor.matmul(ps[:], kt[dt][:, j0:j0 + P],
                                         qt[dt][:, i0:i0 + n],
                                         start=(dt == 0), stop=(dt == DT - 1))
                    nc.scalar.activation(t[:, i0:i0 + n], ps[:], AF.Exp,
                                         bias=pm_sb[:, jb:jb + 1], scale=SCALE)
                    i0 += n
                # mask the diagonal tile: keep j<=i (upper-right triangle)
                nc.vector.tensor_mul(t[:, j0:j0 + P], t[:, j0:j0 + P], tri_sb[:])

            # --- O' = P^T.T @ V, rowsum = P^T.T @ ones; normalize+relu ---
            for ib in range(LT):
                i0 = ib * P
                ops = o_ps.tile([P, D], F32, tag="op", name=f"ops{ib}_{b}")
                rps = r_ps.tile([P, 1], F32, tag="rp", name=f"rps{ib}_{b}")
                for jb in range(ib + 1):
                    pT = pt[jb][:, i0:i0 + P]
                    nc.tensor.matmul(ops[:], pT, v[jb][:],
                                     start=(jb == 0), stop=(jb == ib))
                    nc.tensor.matmul(rps[:], pT, ones_sb[:],
                                     start=(jb == 0), stop=(jb == ib))
                rec = sm_pool.tile([P, 1], F32, tag="rec", name=f"rec{ib}_{b}")
                nc.vector.reciprocal(rec[:], rps[:])
                o_sb = o_pool.tile([P, D], F32, tag="ot", name=f"o{ib}_{b}")
                nc.scalar.activation(o_sb[:], ops[:], AF.Relu, scale=rec[:])
                # SWDGE so stores never head-of-line-block the x prefetch
                nc.gpsimd.dma_start(out[b, i0:i0 + P, :], o_sb[:])

    nc.compile()
    return nc


def _prep_host(x, Wq, bq, Wk, bk, Wv, bv, mask):
    bf = ml_dtypes.bfloat16
    # x transposed to [B, C, L] so on-device loads are plain contiguous DMAs
    xb16 = np.ascontiguousarray(x.astype(bf).transpose(0, 2, 1))
    wqT = np.ascontiguousarray(Wq.T).astype(bf)  # [C, D]
    wkT = np.ascontiguousarray(Wk.T).astype(bf)
    wvT = np.ascontiguousarray(Wv.T).astype(bf)
    bq2 = np.ascontiguousarray(
        bq.astype(np.float32).reshape(DT, P).T)  # [P, DT]
    bk2 = np.ascontiguousarray(bk.astype(np.float32).reshape(DT, P).T)
    bvb = np.ascontiguousarray(
        np.broadcast_to(bv.astype(np.float32), (P, D)))  # [P, D]
    pm = np.where(mask[:, 0, :] != 0, 0.0, NEG).astype(np.float32)  # [B, L]
    pmt = np.ascontiguousarray(
        pm.reshape(B, LT, P).transpose(0, 2, 1))  # [B, P, LT]
    tri = (np.arange(P)[:, None] <= np.arange(P)[None, :]).astype(bf)
    return xb16, wqT, wkT, wvT, bq2, bk2, bvb, pmt, tri


_NC_CACHE = {}


def kernel(x, Wq, bq, Wk, bk, Wv, bv, mask):
    x = np.asarray(x)
    Wq, bq = np.asarray(Wq), np.asarray(bq)
    Wk, bk = np.asarray(Wk), np.asarray(bk)
    Wv, bv = np.asarray(Wv), np.asarray(bv)
    mask = np.asarray(mask)

    xb16, wqT, wkT, wvT, bq2, bk2, bvb, pmt, tri = _prep_host(
        x, Wq, bq, Wk, bk, Wv, bv, mask)

    if "nc" not in _NC_CACHE:
        _NC_CACHE["nc"] = build_program(NB)
    nc = _NC_CACHE["nc"]

    in_maps = []
    for c in range(N_CORES):
        s = slice(c * NB, (c + 1) * NB)
        in_maps.append({
            "xtb": np.ascontiguousarray(xb16[s]),
            "wqT": wqT, "wkT": wkT, "wvT": wvT,
            "bq2": bq2, "bk2": bk2, "bvb": bvb,
            "pmt": np.ascontiguousarray(pmt[s]),
            "tri": tri,
        })

    res = bass_utils.run_bass_kernel_spmd(
        nc, in_maps, core_ids=list(range(N_CORES)),
        trace=bool(int(os.environ.get("KERNEL_TRACE", "0"))),
    )
    if os.environ.get("KERNEL_RESULT_HOOK"):
        _NC_CACHE["last_result"] = res

    return np.concatenate([res.results[c]["out"] for c in range(N_CORES)],
                          axis=0)


# revision 9
# speedup vs baseline: 1.0844x; 1.0078x over previous
"""Causal attention (QKV proj + softmax + PV + ReLU) on 8 trn2 NeuronCores.

Sharding: data-parallel over batch B=32 -> 4 batches per core; projection
weights replicated. Inside each core, per batch:
  X^T via DMA-transpose (bf16)
  Q^T,K^T = W^T.T @ X^T   (bias folded into PSUM->SBUF drain)
  V = X^T.T @ Wv^T        (bias via DVE add during drain)
  S^T[j,i] tiles = K^T.T @ Q^T, causal-sparse (upper j>i tiles skipped)
  P^T = exp(scale*S^T + padmask_bias_j)  (ACT, fused), diag tile masked by mul
  O' = P^T.T @ V and rowsum = P^T.T @ ones ride the same stationary operand
  out = Relu(O' * (1/rowsum))  (one ACT op), DMA out.
"""

import os
from contextlib import ExitStack

import numpy as np
import ml_dtypes

import concourse.bass as bass
import concourse.tile as tile
from concourse import bacc, mybir
from concourse import bass_utils

F32 = mybir.dt.float32
BF16 = mybir.dt.bfloat16
AF = mybir.ActivationFunctionType

N_CORES = 8
B = 32
L = 1024
C = 1024  # d_model
D = 512
P = 128
NB = B // N_CORES  # batches per core
CT = C // P  # 8 contraction tiles
DT = D // P  # 4 d tiles
LT = L // P  # 8 l/j/i tiles
SCALE = float(D) ** -0.5
NEG = -30000.0


def build_program(nb: int = NB):
    """Build the per-core Bass program for nb batches."""
    nc = bacc.Bacc("TRN2", target_bir_lowering=False, debug=False,
                   num_devices=N_CORES)

    xtb = nc.dram_tensor("xtb", [nb, C, L], BF16, kind="ExternalInput").ap()
    wqT = nc.dram_tensor("wqT", [C, D], BF16, kind="ExternalInput").ap()
    wkT = nc.dram_tensor("wkT", [C, D], BF16, kind="ExternalInput").ap()
    wvT = nc.dram_tensor("wvT", [C, D], BF16, kind="ExternalInput").ap()
    bq2 = nc.dram_tensor("bq2", [P, DT], F32, kind="ExternalInput").ap()
    bk2 = nc.dram_tensor("bk2", [P, DT], F32, kind="ExternalInput").ap()
    bvb = nc.dram_tensor("bvb", [P, D], F32, kind="ExternalInput").ap()
    pmt = nc.dram_tensor("pmt", [nb, P, LT], F32, kind="ExternalInput").ap()
    tri = nc.dram_tensor("tri", [P, P], BF16, kind="ExternalInput").ap()
    out = nc.dram_tensor("out", [nb, L, D], F32, kind="ExternalOutput").ap()

    with tile.TileContext(nc) as tc, ExitStack() as ctx:
        const = ctx.enter_context(tc.tile_pool(name="const", bufs=1))
        xt_pool = ctx.enter_context(tc.tile_pool(name="xt", bufs=3))
        qk_pool = ctx.enter_context(tc.tile_pool(name="qk", bufs=2))
        v_pool = ctx.enter_context(tc.tile_pool(name="v", bufs=2))
        pt_pool = ctx.enter_context(tc.tile_pool(name="pt", bufs=2))
        o_pool = ctx.enter_context(tc.tile_pool(name="o", bufs=3))
        sm_pool = ctx.enter_context(tc.tile_pool(name="sm", bufs=4))
        pm_pool = ctx.enter_context(tc.tile_pool(name="pm", bufs=2))
        proj_ps = ctx.enter_context(tc.tile_pool(name="pps", bufs=2, space="PSUM"))
        s_ps = ctx.enter_context(tc.tile_pool(name="sps", bufs=2, space="PSUM"))
        o_ps = ctx.enter_context(tc.tile_pool(name="ops", bufs=2, space="PSUM"))
        r_ps = ctx.enter_context(tc.tile_pool(name="rps", bufs=2, space="PSUM"))

        # --- constants, loaded once; all on the scalar HWDGE queue so the
        # sync queue is dedicated to x prefetch ---
        wq_sb = const.tile([P, CT, D], BF16)
        nc.scalar.dma_start(wq_sb[:], wqT.rearrange("(t p) d -> p t d", p=P))
        wk_sb = const.tile([P, CT, D], BF16)
        nc.scalar.dma_start(wk_sb[:], wkT.rearrange("(t p) d -> p t d", p=P))
        wv_sb = const.tile([P, CT, D], BF16)
        nc.scalar.dma_start(wv_sb[:], wvT.rearrange("(t p) d -> p t d", p=P))
        bq_sb = const.tile([P, DT], F32)
        nc.scalar.dma_start(bq_sb[:], bq2[:])
        bk_sb = const.tile([P, DT], F32)
        nc.scalar.dma_start(bk_sb[:], bk2[:])
        bv_sb = const.tile([P, D], F32)
        nc.scalar.dma_start(bv_sb[:], bvb[:])
        tri_sb = const.tile([P, P], BF16)
        nc.scalar.dma_start(tri_sb[:], tri[:])
        ones_sb = const.tile([P, 1], BF16)
        nc.vector.memset(ones_sb[:], 1.0)

        for b in range(nb):
            # --- X^T tiles [128c, 1024l], pretransposed on host ---
            xt = []
            for ct in range(CT):
                t = xt_pool.tile([P, L], BF16, tag=f"xt{ct}", name=f"xt{ct}_{b}")
                nc.sync.dma_start(t[:], xtb[b, ct * P:(ct + 1) * P, :])
                xt.append(t)
            pm_sb = pm_pool.tile([P, LT], F32, name=f"pm_{b}")
            nc.sync.dma_start(pm_sb[:], pmt[b])

            # --- Q^T, K^T: [128d, 1024l] tiles; bias added at drain ---
            qt, kt = [], []
            for name, w_sb, b_sb, dst in (("q", wq_sb, bq_sb, qt),
                                          ("k", wk_sb, bk_sb, kt)):
                for dt in range(DT):
                    t = qk_pool.tile([P, L], BF16, tag=f"{name}t{dt}",
                                     name=f"{name}t{dt}_{b}")
                    dst.append(t)
                    for lc in range(L // 512):
                        ps = proj_ps.tile([P, 512], F32, tag="pp",
                                          name=f"{name}ps{dt}_{lc}_{b}")
                        for ct in range(CT):
                            nc.tensor.matmul(
                                ps[:],
                                w_sb[:, ct, dt * P:(dt + 1) * P],
                                xt[ct][:, lc * 512:(lc + 1) * 512],
                                start=(ct == 0), stop=(ct == CT - 1))
                        nc.vector.tensor_scalar_add(
                            t[:, lc * 512:(lc + 1) * 512], ps[:],
                            b_sb[:, dt:dt + 1])

            # --- V: [128l, 512d] tiles; bias via DVE add at drain ---
            v = []
            for lt in range(LT):
                t = v_pool.tile([P, D], BF16, tag=f"v{lt}", name=f"v{lt}_{b}")
                v.append(t)
                ps = proj_ps.tile([P, D], F32, tag="pp", name=f"vps{lt}_{b}")
                for ct in range(CT):
                    nc.tensor.matmul(ps[:], xt[ct][:, lt * P:(lt + 1) * P],
                                     wv_sb[:, ct, :],
                                     start=(ct == 0), stop=(ct == CT - 1))
                nc.vector.tensor_add(t[:], ps[:], bv_sb[:])

            # --- S^T tiles + exp -> P^T (causal: only i >= j0 computed) ---
            pt = []
            for jb in range(LT):
                j0 = jb * P
                t = pt_pool.tile([P, L], BF16, tag=f"pt{jb}", name=f"pt{jb}_{b}")
                pt.append(t)
                i0 = j0
                while i0 < L:
                    n = min((i0 // 512 + 1) * 512, L) - i0
                    ps = s_ps.tile([P, n], F32, tag="sp", name=f"sps{jb}_{i0}_{b}")
                    for dt in range(DT):
                        nc.tensor.matmul(ps[:], kt[dt][:, j0:j0 + P],
                                         qt[dt][:, i0:i0 + n],
                                         start=(dt == 0), stop=(dt == DT - 1))
                    nc.scalar.activation(t[:, i0:i0 + n], ps[:], AF.Exp,
                                         bias=pm_sb[:, jb:jb + 1], scale=SCALE)
                    i0 += n
                # mask the diagonal tile: keep j<=i (upper-right triangle)
                nc.vector.tensor_mul(t[:, j0:j0 + P], t[:, j0:j0 + P], tri_sb[:])

            # --- O' = P^T.T @ V, rowsum = P^T.T @ ones; normalize+relu ---
            for ib in range(LT):
                i0 = ib * P
                ops = o_ps.tile([P, D], F32, tag="op", name=f"ops{ib}_{b}")
                rps = r_ps.tile([P, 1], F32, tag="rp", name=f"rps{ib}_{b}")
                for jb in range(ib + 1):
                    pT = pt[jb][:, i0:i0 + P]
                    nc.tensor.matmul(ops[:], pT, v[jb][:],
                                     start=(jb == 0), stop=(jb == ib))
                    nc.tensor.matmul(rps[:], pT, ones_sb[:],
                                     start=(jb == 0), stop=(jb == ib))
                rec = sm_pool.tile([P, 1], F32, tag="rec", name=f"rec{ib}_{b}")
                nc.vector.reciprocal(rec[:], rps[:])
                o_sb = o_pool.tile([P, D], F32, tag="ot", name=f"o{ib}_{b}")
                nc.scalar.activation(o_sb[:], ops[:], AF.Relu, scale=rec[:])
                # SWDGE so stores never head-of-line-block the x prefetch
                nc.gpsimd.dma_start(out[b, i0:i0 + P, :], o_sb[:])

    nc.compile()
    return nc


def _prep_host(x, Wq, bq, Wk, bk, Wv, bv, mask):
    bf = ml_dtypes.bfloat16
    # x transposed to [B, C, L] so on-device loads are plain contiguous DMAs
    xb16 = np.ascontiguousarray(x.astype(bf).transpose(0, 2, 1))
    wqT = np.ascontiguousarray(Wq.T).astype(bf)  # [C, D]
    wkT = np.ascontiguousarray(Wk.T).astype(bf)
    wvT = np.ascontiguousarray(Wv.T).astype(bf)
    bq2 = np.ascontiguousarray(
        bq.astype(np.float32).reshape(DT, P).T)  # [P, DT]
    bk2 = np.ascontiguousarray(bk.astype(np.float32).reshape(DT, P).T)
    bvb = np.ascontiguousarray(
        np.broadcast_to(bv.astype(np.float32), (P, D)))  # [P, D]
    pm = np.where(mask[:, 0, :] != 0, 0.0, NEG).astype(np.float32)  # [B, L]
    pmt = np.ascontiguousarray(
        pm.reshape(B, LT, P).transpose(0, 2, 1))  # [B, P, LT]
    tri = (np.arange(P)[:, None] <= np.arange(P)[None, :]).astype(bf)
    return xb16, wqT, wkT, wvT, bq2, bk2, bvb, pmt, tri


_NC_CACHE = {}


def kernel(x, Wq, bq, Wk, bk, Wv, bv, mask):
    x = np.asarray(x)
    Wq, bq = np.asarray(Wq), np.asarray(bq)
    Wk, bk = np.asarray(Wk), np.asarray(bk)
    Wv, bv = np.asarray(Wv), np.asarray(bv)
    mask = np.asarray(mask)

    xb16, wqT, wkT, wvT, bq2, bk2, bvb, pmt, tri = _prep_host(
        x, Wq, bq, Wk, bk, Wv, bv, mask)

    if "nc" not in _NC_CACHE:
        _NC_CACHE["nc"] = build_program(NB)
    nc = _NC_CACHE["nc"]

    in_maps = []
    for c in range(N_CORES):
        s = slice(c * NB, (c + 1) * NB)
        in_maps.append({
            "xtb": np.ascontiguousarray(xb16[s]),
            "wqT": wqT, "wkT": wkT, "wvT": wvT,
            "bq2": bq2, "bk2": bk2, "bvb": bvb,
            "pmt": np.ascontiguousarray(pmt[s]),
            "tri": tri,
        })

    res = bass_utils.run_bass_kernel_spmd(
        nc, in_maps, core_ids=list(range(N_CORES)),
        trace=bool(int(os.environ.get("KERNEL_TRACE", "0"))),
    )
    if os.environ.get("KERNEL_RESULT_HOOK"):
        _NC_CACHE["last_result"] = res

    return np.concatenate([res.results[c]["out"] for c in range(N_CORES)],
                          axis=0)


# revision 10
# speedup vs baseline: 1.1048x; 1.0189x over previous
"""Causal attention (QKV proj + softmax + PV + ReLU) on 8 trn2 NeuronCores.

Sharding: data-parallel over batch B=32 -> 4 batches per core; projection
weights replicated. Inside each core, per batch:
  X^T via DMA-transpose (bf16)
  Q^T,K^T = W^T.T @ X^T   (bias folded into PSUM->SBUF drain)
  V = X^T.T @ Wv^T        (bias via DVE add during drain)
  S^T[j,i] tiles = K^T.T @ Q^T, causal-sparse (upper j>i tiles skipped)
  P^T = exp(scale*S^T + padmask_bias_j)  (ACT, fused), diag tile masked by mul
  O' = P^T.T @ V and rowsum = P^T.T @ ones ride the same stationary operand
  out = Relu(O' * (1/rowsum))  (one ACT op), DMA out.
"""

import os
from contextlib import ExitStack

import numpy as np
import ml_dtypes

import concourse.bass as bass
import concourse.tile as tile
from concourse import bacc, mybir
from concourse import bass_utils

F32 = mybir.dt.float32
F16 = mybir.dt.float16
AF = mybir.ActivationFunctionType

N_CORES = 8
B = 32
L = 1024
C = 1024  # d_model
D = 512
P = 128
NB = B // N_CORES  # batches per core
CT = C // P  # 8 contraction tiles
DT = D // P  # 4 d tiles
LT = L // P  # 8 l/j/i tiles
SCALE = float(D) ** -0.5
NEG = -30000.0


def build_program(nb: int = NB):
    """Build the per-core Bass program for nb batches."""
    nc = bacc.Bacc("TRN2", target_bir_lowering=False, debug=False,
                   num_devices=N_CORES)

    xtb = nc.dram_tensor("xtb", [nb, C, L], F16, kind="ExternalInput").ap()
    wqT = nc.dram_tensor("wqT", [C, D], F16, kind="ExternalInput").ap()
    wkT = nc.dram_tensor("wkT", [C, D], F16, kind="ExternalInput").ap()
    wvT = nc.dram_tensor("wvT", [C, D], F16, kind="ExternalInput").ap()
    bq2 = nc.dram_tensor("bq2", [P, DT], F32, kind="ExternalInput").ap()
    bk2 = nc.dram_tensor("bk2", [P, DT], F32, kind="ExternalInput").ap()
    bvb = nc.dram_tensor("bvb", [P, D], F32, kind="ExternalInput").ap()
    pmt = nc.dram_tensor("pmt", [nb, P, LT], F32, kind="ExternalInput").ap()
    tri = nc.dram_tensor("tri", [P, P], F16, kind="ExternalInput").ap()
    out = nc.dram_tensor("out", [nb, L, D], F32, kind="ExternalOutput").ap()

    with tile.TileContext(nc) as tc, ExitStack() as ctx:
        const = ctx.enter_context(tc.tile_pool(name="const", bufs=1))
        xt_pool = ctx.enter_context(tc.tile_pool(name="xt", bufs=3))
        qk_pool = ctx.enter_context(tc.tile_pool(name="qk", bufs=2))
        v_pool = ctx.enter_context(tc.tile_pool(name="v", bufs=2))
        pt_pool = ctx.enter_context(tc.tile_pool(name="pt", bufs=2))
        o_pool = ctx.enter_context(tc.tile_pool(name="o", bufs=3))
        sm_pool = ctx.enter_context(tc.tile_pool(name="sm", bufs=4))
        pm_pool = ctx.enter_context(tc.tile_pool(name="pm", bufs=2))
        proj_ps = ctx.enter_context(tc.tile_pool(name="pps", bufs=2, space="PSUM"))
        s_ps = ctx.enter_context(tc.tile_pool(name="sps", bufs=2, space="PSUM"))
        o_ps = ctx.enter_context(tc.tile_pool(name="ops", bufs=2, space="PSUM"))
        r_ps = ctx.enter_context(tc.tile_pool(name="rps", bufs=2, space="PSUM"))

        # --- constants, loaded once; all on the scalar HWDGE queue so the
        # sync queue is dedicated to x prefetch ---
        wq_sb = const.tile([P, CT, D], F16)
        nc.scalar.dma_start(wq_sb[:], wqT.rearrange("(t p) d -> p t d", p=P))
        wk_sb = const.tile([P, CT, D], F16)
        wv_sb = const.tile([P, CT, D], F16)
        bq_sb = const.tile([P, DT], F32)
        nc.scalar.dma_start(bq_sb[:], bq2[:])
        bk_sb = const.tile([P, DT], F32)
        nc.scalar.dma_start(bk_sb[:], bk2[:])
        bv_sb = const.tile([P, D], F32)
        nc.scalar.dma_start(bv_sb[:], bvb[:])
        tri_sb = const.tile([P, P], F16)
        nc.scalar.dma_start(tri_sb[:], tri[:])
        ones_sb = const.tile([P, 1], F16)
        nc.vector.memset(ones_sb[:], 1.0)

        for b in range(nb):
            # --- X^T tiles [128c, 1024l], pretransposed on host ---
            xt = []
            for ct in range(CT):
                t = xt_pool.tile([P, L], F16, tag=f"xt{ct}", name=f"xt{ct}_{b}")
                nc.sync.dma_start(t[:], xtb[b, ct * P:(ct + 1) * P, :])
                xt.append(t)
            pm_sb = pm_pool.tile([P, LT], F32, name=f"pm_{b}")
            nc.sync.dma_start(pm_sb[:], pmt[b])
            if b == 0:
                # behind batch-0 x in the sync ring: wk/wv don't steal HBM
                # bandwidth from the startup-critical x prefetch
                nc.sync.dma_start(wk_sb[:],
                                  wkT.rearrange("(t p) d -> p t d", p=P))
                nc.sync.dma_start(wv_sb[:],
                                  wvT.rearrange("(t p) d -> p t d", p=P))

            # --- Q^T, K^T: [128d, 1024l] tiles; bias added at drain ---
            qt, kt = [], []
            for name, w_sb, b_sb, dst in (("q", wq_sb, bq_sb, qt),
                                          ("k", wk_sb, bk_sb, kt)):
                for dt in range(DT):
                    t = qk_pool.tile([P, L], F16, tag=f"{name}t{dt}",
                                     name=f"{name}t{dt}_{b}")
                    dst.append(t)
                    for lc in range(L // 512):
                        ps = proj_ps.tile([P, 512], F32, tag="pp",
                                          name=f"{name}ps{dt}_{lc}_{b}")
                        for ct in range(CT):
                            nc.tensor.matmul(
                                ps[:],
                                w_sb[:, ct, dt * P:(dt + 1) * P],
                                xt[ct][:, lc * 512:(lc + 1) * 512],
                                start=(ct == 0), stop=(ct == CT - 1))
                        nc.vector.tensor_scalar_add(
                            t[:, lc * 512:(lc + 1) * 512], ps[:],
                            b_sb[:, dt:dt + 1])

            # --- V: [128l, 512d] tiles; bias via DVE add at drain ---
            v = []
            for lt in range(LT):
                t = v_pool.tile([P, D], F16, tag=f"v{lt}", name=f"v{lt}_{b}")
                v.append(t)
                ps = proj_ps.tile([P, D], F32, tag="pp", name=f"vps{lt}_{b}")
                for ct in range(CT):
                    nc.tensor.matmul(ps[:], xt[ct][:, lt * P:(lt + 1) * P],
                                     wv_sb[:, ct, :],
                                     start=(ct == 0), stop=(ct == CT - 1))
                nc.vector.tensor_add(t[:], ps[:], bv_sb[:])

            # --- S^T tiles + exp -> P^T (causal: only i >= j0 computed) ---
            pt = []
            for jb in range(LT):
                j0 = jb * P
                t = pt_pool.tile([P, L], F16, tag=f"pt{jb}", name=f"pt{jb}_{b}")
                pt.append(t)
                i0 = j0
                while i0 < L:
                    n = min((i0 // 512 + 1) * 512, L) - i0
                    ps = s_ps.tile([P, n], F32, tag="sp", name=f"sps{jb}_{i0}_{b}")
                    for dt in range(DT):
                        nc.tensor.matmul(ps[:], kt[dt][:, j0:j0 + P],
                                         qt[dt][:, i0:i0 + n],
                                         start=(dt == 0), stop=(dt == DT - 1))
                    nc.scalar.activation(t[:, i0:i0 + n], ps[:], AF.Exp,
                                         bias=pm_sb[:, jb:jb + 1], scale=SCALE)
                    i0 += n
                # mask the diagonal tile: keep j<=i (upper-right triangle)
                nc.vector.tensor_mul(t[:, j0:j0 + P], t[:, j0:j0 + P], tri_sb[:])

            # --- O' = P^T.T @ V, rowsum = P^T.T @ ones; normalize+relu ---
            for ib in range(LT):
                i0 = ib * P
                ops = o_ps.tile([P, D], F32, tag="op", name=f"ops{ib}_{b}")
                rps = r_ps.tile([P, 1], F32, tag="rp", name=f"rps{ib}_{b}")
                for jb in range(ib + 1):
                    pT = pt[jb][:, i0:i0 + P]
                    nc.tensor.matmul(ops[:], pT, v[jb][:],
                                     start=(jb == 0), stop=(jb == ib))
                    nc.tensor.matmul(rps[:], pT, ones_sb[:],
                                     start=(jb == 0), stop=(jb == ib))
                rec = sm_pool.tile([P, 1], F32, tag="rec", name=f"rec{ib}_{b}")
                nc.vector.reciprocal(rec[:], rps[:])
                o_sb = o_pool.tile([P, D], F32, tag="ot", name=f"o{ib}_{b}")
                nc.scalar.activation(o_sb[:], ops[:], AF.Relu, scale=rec[:])
                # SWDGE so stores never head-of-line-block the x prefetch
                nc.gpsimd.dma_start(out[b, i0:i0 + P, :], o_sb[:])

    nc.compile()
    return nc


def _prep_host(x, Wq, bq, Wk, bk, Wv, bv, mask):
    bf = np.float16
    # x transposed to [B, C, L] so on-device loads are plain contiguous DMAs
    xb16 = np.ascontiguousarray(x.astype(bf).transpose(0, 2, 1))
    wqT = np.ascontiguousarray(Wq.T).astype(bf)  # [C, D]
    wkT = np.ascontiguousarray(Wk.T).astype(bf)
    wvT = np.ascontiguousarray(Wv.T).astype(bf)
    bq2 = np.ascontiguousarray(
        bq.astype(np.float32).reshape(DT, P).T)  # [P, DT]
    bk2 = np.ascontiguousarray(bk.astype(np.float32).reshape(DT, P).T)
    bvb = np.ascontiguousarray(
        np.broadcast_to(bv.astype(np.float32), (P, D)))  # [P, D]
    pm = np.where(mask[:, 0, :] != 0, 0.0, NEG).astype(np.float32)  # [B, L]
    pmt = np.ascontiguousarray(
        pm.reshape(B, LT, P).transpose(0, 2, 1))  # [B, P, LT]
    tri = (np.arange(P)[:, None] <= np.arange(P)[None, :]).astype(bf)
    return xb16, wqT, wkT, wvT, bq2, bk2, bvb, pmt, tri


_NC_CACHE = {}


def kernel(x, Wq, bq, Wk, bk, Wv, bv, mask):
    x = np.asarray(x)
    Wq, bq = np.asarray(Wq), np.asarray(bq)
    Wk, bk = np.asarray(Wk), np.asarray(bk)
    Wv, bv = np.asarray(Wv), np.asarray(bv)
    mask = np.asarray(mask)

    xb16, wqT, wkT, wvT, bq2, bk2, bvb, pmt, tri = _prep_host(
        x, Wq, bq, Wk, bk, Wv, bv, mask)

    if "nc" not in _NC_CACHE:
        _NC_CACHE["nc"] = build_program(NB)
    nc = _NC_CACHE["nc"]

    in_maps = []
    for c in range(N_CORES):
        s = slice(c * NB, (c + 1) * NB)
        in_maps.append({
            "xtb": np.ascontiguousarray(xb16[s]),
            "wqT": wqT, "wkT": wkT, "wvT": wvT,
            "bq2": bq2, "bk2": bk2, "bvb": bvb,
            "pmt": np.ascontiguousarray(pmt[s]),
            "tri": tri,
        })

    res = bass_utils.run_bass_kernel_spmd(
        nc, in_maps, core_ids=list(range(N_CORES)),
        trace=bool(int(os.environ.get("KERNEL_TRACE", "0"))),
    )
    if os.environ.get("KERNEL_RESULT_HOOK"):
        _NC_CACHE["last_result"] = res

    return np.concatenate([res.results[c]["out"] for c in range(N_CORES)],
                          axis=0)


# revision 12
# speedup vs baseline: 1.1162x; 1.0103x over previous
"""Causal attention (QKV proj + softmax + PV + ReLU) on 8 trn2 NeuronCores.

Sharding: data-parallel over batch B=32 -> 4 batches per core; projection
weights replicated. Inside each core, per batch:
  X^T via DMA-transpose (bf16)
  Q^T,K^T = W^T.T @ X^T   (bias folded into PSUM->SBUF drain)
  V = X^T.T @ Wv^T        (bias via DVE add during drain)
  S^T[j,i] tiles = K^T.T @ Q^T, causal-sparse (upper j>i tiles skipped)
  P^T = exp(scale*S^T + padmask_bias_j)  (ACT, fused), diag tile masked by mul
  O' = P^T.T @ V and rowsum = P^T.T @ ones ride the same stationary operand
  out = Relu(O' * (1/rowsum))  (one ACT op), DMA out.
"""

import os
from contextlib import ExitStack

import numpy as np
import ml_dtypes

import concourse.bass as bass
import concourse.tile as tile
from concourse import bacc, mybir
from concourse import bass_utils

F32 = mybir.dt.float32
F16 = mybir.dt.float16
AF = mybir.ActivationFunctionType

N_CORES = 8
B = 32
L = 1024
C = 1024  # d_model
D = 512
P = 128
NB = B // N_CORES  # batches per core
CT = C // P  # 8 contraction tiles
DT = D // P  # 4 d tiles
LT = L // P  # 8 l/j/i tiles
SCALE = float(D) ** -0.5
NEG = -30000.0


def build_program(nb: int = NB):
    """Build the per-core Bass program for nb batches."""
    nc = bacc.Bacc("TRN2", target_bir_lowering=False, debug=False,
                   num_devices=N_CORES)

    xtb = nc.dram_tensor("xtb", [nb, C, L], F16, kind="ExternalInput").ap()
    wqT = nc.dram_tensor("wqT", [C, D], F16, kind="ExternalInput").ap()
    wkT = nc.dram_tensor("wkT", [C, D], F16, kind="ExternalInput").ap()
    wvT = nc.dram_tensor("wvT", [C, D], F16, kind="ExternalInput").ap()
    bq2 = nc.dram_tensor("bq2", [P, DT], F32, kind="ExternalInput").ap()
    bk2 = nc.dram_tensor("bk2", [P, DT], F32, kind="ExternalInput").ap()
    bvb = nc.dram_tensor("bvb", [P, D], F32, kind="ExternalInput").ap()
    pmt = nc.dram_tensor("pmt", [nb, P, LT], F32, kind="ExternalInput").ap()
    tri = nc.dram_tensor("tri", [P, P], F16, kind="ExternalInput").ap()
    out = nc.dram_tensor("out", [nb, L, D], F32, kind="ExternalOutput").ap()

    with tile.TileContext(nc) as tc, ExitStack() as ctx:
        const = ctx.enter_context(tc.tile_pool(name="const", bufs=1))
        xt_pool = ctx.enter_context(tc.tile_pool(name="xt", bufs=3))
        qk_pool = ctx.enter_context(tc.tile_pool(name="qk", bufs=2))
        v_pool = ctx.enter_context(tc.tile_pool(name="v", bufs=2))
        pt_pool = ctx.enter_context(tc.tile_pool(name="pt", bufs=2))
        o_pool = ctx.enter_context(tc.tile_pool(name="o", bufs=3))
        sm_pool = ctx.enter_context(tc.tile_pool(name="sm", bufs=4))
        pm_pool = ctx.enter_context(tc.tile_pool(name="pm", bufs=2))
        proj_ps = ctx.enter_context(tc.tile_pool(name="pps", bufs=2, space="PSUM"))
        s_ps = ctx.enter_context(tc.tile_pool(name="sps", bufs=2, space="PSUM"))
        o_ps = ctx.enter_context(tc.tile_pool(name="ops", bufs=2, space="PSUM"))
        r_ps = ctx.enter_context(tc.tile_pool(name="rps", bufs=2, space="PSUM"))

        # --- constants, loaded once; all on the scalar HWDGE queue so the
        # sync queue is dedicated to x prefetch ---
        wq_sb = const.tile([P, CT, D], F16)
        nc.scalar.dma_start(wq_sb[:], wqT.rearrange("(t p) d -> p t d", p=P))
        wk_sb = const.tile([P, CT, D], F16)
        wv_sb = const.tile([P, CT, D], F16)
        bq_sb = const.tile([P, DT], F32)
        nc.scalar.dma_start(bq_sb[:], bq2[:])
        bk_sb = const.tile([P, DT], F32)
        nc.scalar.dma_start(bk_sb[:], bk2[:])
        bv_sb = const.tile([P, D], F32)
        nc.scalar.dma_start(bv_sb[:], bvb[:])
        tri_sb = const.tile([P, P], F16)
        nc.scalar.dma_start(tri_sb[:], tri[:])
        ones_sb = const.tile([P, 1], F16)
        nc.vector.memset(ones_sb[:], 1.0)

        # PE warmup: dummy matmuls with no input deps keep the PE busy while
        # batch-0 inputs stream in, so the HAM clock-gate is already at
        # 2.4 GHz when the real stream starts (saves ~4us of cold-clock).
        warm_sb = const.tile([P, 512], F16)
        nc.vector.memset(warm_sb[:], 0.0)
        for w in range(15):
            wps = proj_ps.tile([P, 512], F32, tag="pp", name=f"warm{w}")
            nc.tensor.matmul(wps[:], warm_sb[:, 0:P], warm_sb[:],
                             start=True, stop=True)

        for b in range(nb):
            # --- X^T tiles [128c, 1024l], pretransposed on host ---
            xt = []
            for ct in range(CT):
                t = xt_pool.tile([P, L], F16, tag=f"xt{ct}", name=f"xt{ct}_{b}")
                nc.sync.dma_start(t[:], xtb[b, ct * P:(ct + 1) * P, :])
                xt.append(t)
            pm_sb = pm_pool.tile([P, LT], F32, name=f"pm_{b}")
            nc.sync.dma_start(pm_sb[:], pmt[b])
            if b == 0:
                # behind batch-0 x in the sync ring: wk/wv don't steal HBM
                # bandwidth from the startup-critical x prefetch
                nc.sync.dma_start(wk_sb[:],
                                  wkT.rearrange("(t p) d -> p t d", p=P))
                nc.sync.dma_start(wv_sb[:],
                                  wvT.rearrange("(t p) d -> p t d", p=P))

            # --- Q^T, K^T: [128d, 1024l] tiles; bias added at drain ---
            qt, kt = [], []
            for name, w_sb, b_sb, dst in (("q", wq_sb, bq_sb, qt),
                                          ("k", wk_sb, bk_sb, kt)):
                for dt in range(DT):
                    t = qk_pool.tile([P, L], F16, tag=f"{name}t{dt}",
                                     name=f"{name}t{dt}_{b}")
                    dst.append(t)
                    for lc in range(L // 512):
                        ps = proj_ps.tile([P, 512], F32, tag="pp",
                                          name=f"{name}ps{dt}_{lc}_{b}")
                        for ct in range(CT):
                            nc.tensor.matmul(
                                ps[:],
                                w_sb[:, ct, dt * P:(dt + 1) * P],
                                xt[ct][:, lc * 512:(lc + 1) * 512],
                                start=(ct == 0), stop=(ct == CT - 1))
                        nc.vector.tensor_scalar_add(
                            t[:, lc * 512:(lc + 1) * 512], ps[:],
                            b_sb[:, dt:dt + 1])

            # --- V: [128l, 512d] tiles; bias via DVE add at drain ---
            v = []
            for lt in range(LT):
                t = v_pool.tile([P, D], F16, tag=f"v{lt}", name=f"v{lt}_{b}")
                v.append(t)
                ps = proj_ps.tile([P, D], F32, tag="pp", name=f"vps{lt}_{b}")
                for ct in range(CT):
                    nc.tensor.matmul(ps[:], xt[ct][:, lt * P:(lt + 1) * P],
                                     wv_sb[:, ct, :],
                                     start=(ct == 0), stop=(ct == CT - 1))
                nc.vector.tensor_add(t[:], ps[:], bv_sb[:])

            # --- S^T tiles + exp -> P^T (causal: only i >= j0 computed) ---
            pt = []
            for jb in range(LT):
                j0 = jb * P
                t = pt_pool.tile([P, L], F16, tag=f"pt{jb}", name=f"pt{jb}_{b}")
                pt.append(t)
                i0 = j0
                while i0 < L:
                    n = min((i0 // 512 + 1) * 512, L) - i0
                    ps = s_ps.tile([P, n], F32, tag="sp", name=f"sps{jb}_{i0}_{b}")
                    for dt in range(DT):
                        nc.tensor.matmul(ps[:], kt[dt][:, j0:j0 + P],
                                         qt[dt][:, i0:i0 + n],
                                         start=(dt == 0), stop=(dt == DT - 1))
                    nc.scalar.activation(t[:, i0:i0 + n], ps[:], AF.Exp,
                                         bias=pm_sb[:, jb:jb + 1], scale=SCALE)
                    i0 += n
                # mask the diagonal tile: keep j<=i (upper-right triangle)
                nc.vector.tensor_mul(t[:, j0:j0 + P], t[:, j0:j0 + P], tri_sb[:])

            # --- O' = P^T.T @ V, rowsum = P^T.T @ ones; normalize+relu ---
            for ib in range(LT):
                i0 = ib * P
                ops = o_ps.tile([P, D], F32, tag="op", name=f"ops{ib}_{b}")
                rps = r_ps.tile([P, 1], F32, tag="rp", name=f"rps{ib}_{b}")
                for jb in range(ib + 1):
                    pT = pt[jb][:, i0:i0 + P]
                    nc.tensor.matmul(ops[:], pT, v[jb][:],
                                     start=(jb == 0), stop=(jb == ib))
                    nc.tensor.matmul(rps[:], pT, ones_sb[:],
                                     start=(jb == 0), stop=(jb == ib))
                rec = sm_pool.tile([P, 1], F32, tag="rec", name=f"rec{ib}_{b}")
                nc.vector.reciprocal(rec[:], rps[:])
                o_sb = o_pool.tile([P, D], F32, tag="ot", name=f"o{ib}_{b}")
                nc.scalar.activation(o_sb[:], ops[:], AF.Relu, scale=rec[:])
                # SWDGE so stores never head-of-line-block the x prefetch;
                # last batch has no prefetch left, so use the faster HWDGE
                if b == nb - 1:
                    nc.sync.dma_start(out[b, i0:i0 + P, :], o_sb[:])
                else:
                    nc.gpsimd.dma_start(out[b, i0:i0 + P, :], o_sb[:])

    nc.compile()
    return nc


def _prep_host(x, Wq, bq, Wk, bk, Wv, bv, mask):
    bf = np.float16
    # x transposed to [B, C, L] so on-device loads are plain contiguous DMAs
    xb16 = np.ascontiguousarray(x.astype(bf).transpose(0, 2, 1))
    wqT = np.ascontiguousarray(Wq.T).astype(bf)  # [C, D]
    wkT = np.ascontiguousarray(Wk.T).astype(bf)
    wvT = np.ascontiguousarray(Wv.T).astype(bf)
    bq2 = np.ascontiguousarray(
        bq.astype(np.float32).reshape(DT, P).T)  # [P, DT]
    bk2 = np.ascontiguousarray(bk.astype(np.float32).reshape(DT, P).T)
    bvb = np.ascontiguousarray(
        np.broadcast_to(bv.astype(np.float32), (P, D)))  # [P, D]
    pm = np.where(mask[:, 0, :] != 0, 0.0, NEG).astype(np.float32)  # [B, L]
    pmt = np.ascontiguousarray(
        pm.reshape(B, LT, P).transpose(0, 2, 1))  # [B, P, LT]
    tri = (np.arange(P)[:, None] <= np.arange(P)[None, :]).astype(bf)
    return xb16, wqT, wkT, wvT, bq2, bk2, bvb, pmt, tri


_NC_CACHE = {}


def kernel(x, Wq, bq, Wk, bk, Wv, bv, mask):
    x = np.asarray(x)
    Wq, bq = np.asarray(Wq), np.asarray(bq)
    Wk, bk = np.asarray(Wk), np.asarray(bk)
    Wv, bv = np.asarray(Wv), np.asarray(bv)
    mask = np.asarray(mask)

    xb16, wqT, wkT, wvT, bq2, bk2, bvb, pmt, tri = _prep_host(
        x, Wq, bq, Wk, bk, Wv, bv, mask)

    if "nc" not in _NC_CACHE:
        _NC_CACHE["nc"] = build_program(NB)
    nc = _NC_CACHE["nc"]

    in_maps = []
    for c in range(N_CORES):
        s = slice(c * NB, (c + 1) * NB)
        in_maps.append({
            "xtb": np.ascontiguousarray(xb16[s]),
            "wqT": wqT, "wkT": wkT, "wvT": wvT,
            "bq2": bq2, "bk2": bk2, "bvb": bvb,
            "pmt": np.ascontiguousarray(pmt[s]),
            "tri": tri,
        })

    res = bass_utils.run_bass_kernel_spmd(
        nc, in_maps, core_ids=list(range(N_CORES)),
        trace=bool(int(os.environ.get("KERNEL_TRACE", "0"))),
    )
    if os.environ.get("KERNEL_RESULT_HOOK"):
        _NC_CACHE["last_result"] = res

    return np.concatenate([res.results[c]["out"] for c in range(N_CORES)],
                          axis=0)
